# revision 1
# baseline (speedup 1.0000x reference)
"""Multi-head causal attention (B=2, S=2048, D=1024, H=16) on 8 Trainium2
NeuronCores.

Sharding: tensor-parallel over heads - 2 heads per core. Each core computes
its heads' Q/K/V projections, causal attention, and a partial output
projection (row-parallel over the head dims); the host sums the 8 partials
and adds the output bias.

Device layout is fully "transposed" (features on partitions, tokens on the
free axis):
  - QKV projection:  QKVT[f, t]  via lhsT=W^T tiles, rhs=X^T tiles
  - V is immediately PE-transposed to token-major and packed (with a ones
    column) into `vaug` for the attn@V matmuls; Q^T/K^T stay feature-major
  - scores^T[k, q] = KT_tile^T @ QT_block   (contraction = head dim 64)
  - softmax along k (= partitions): exp on ACT, causal mask via mask-mult,
    sums ride the attn@V matmul through the ones column of vaug
  - attn^T[hd, q] accumulates over k tiles in PSUM; normalized by a
    reciprocal broadcast (K=1 matmul) + DVE multiply
  - out^T[e, t] partial = WoutT^T @ attnT, summed across cores on the host

All matmul operands are float32r (full PE rate at N=512, ~1.5e-4 relative
error); accumulation is fp32 in PSUM. Emission order interleaves the later
QKV token-blocks into the first batch's attention stream so the PE always
has independent work while ACT runs exp and the DMAs stream X^T/out.
"""

import contextlib

import numpy as np

import bass_rust
import concourse.bass as bass
import concourse.mybir as mybir
from concourse.bass_utils import run_bass_kernel_spmd
from concourse.tile import TileContext
from concourse.masks import make_identity

F32 = mybir.dt.float32
F32R = mybir.dt.float32r

B, S, D, H = 2, 2048, 1024, 16
HD = D // H            # 64
NCORES = 8
HPC = H // NCORES      # heads per core = 2
DSL = HPC * HD         # feature slice per core = 128
T = B * S              # 4096 tokens
NT = T // 512          # 8 token blocks of 512
ND = D // 128          # 8 d-tiles
NKT = S // 128         # 16 k-tiles per batch
NQB = S // 512         # 4 q-blocks per batch
NSB = NT // 2          # 4 super-blocks of 1024 tokens (1 MiB DMA chunks)


def _split_multi_waits(nc):
    """This walrus build accepts only ONE sync-wait per instruction. Hoist
    all-but-one wait of any multi-wait instruction onto same-engine NoOps
    placed immediately before it (engine program order preserves
    semantics)."""
    n = 0
    for f in nc.m.functions:
        for blk in f.blocks:
            il = blk.instructions
            new = []
            changed = False
            for inst in il:
                si = inst.sync_info
                waits = list(si.on_wait) if si is not None and si.on_wait else []
                if len(waits) > 1:
                    changed = True
                    for w in waits[:-1]:
                        nop = mybir.InstNoOp(
                            name=f"I-waitsplit-{nc.next_id()}", ins=[], outs=[]
                        )
                        nop.engine = inst.engine
                        nop.sync_info = bass_rust.SyncInfo(on_wait=[w], on_update=[])
                        new.append(nop)
                        n += 1
                    inst.sync_info = bass_rust.SyncInfo(
                        on_wait=[waits[-1]], on_update=list(si.on_update or [])
                    )
                new.append(inst)
            if changed:
                blk.instructions = new
    return n


def _build(loop_n=None, loop_phase=None):
    nc = bass.Bass("TRN2", target_bir_lowering=False, debug=False)

    XT = nc.declare_dram_parameter("XT", [D, T], F32R, isOutput=False)
    WQKVT = nc.declare_dram_parameter("WQKVT", [D, 3 * DSL], F32R, isOutput=False)
    BQKV = nc.declare_dram_parameter("BQKV", [3 * DSL, 1], F32, isOutput=False)
    WOUTT = nc.declare_dram_parameter("WOUTT", [DSL, D], F32R, isOutput=False)
    OUTT = nc.declare_dram_parameter("OUTT", [D, T], F32, isOutput=True)

    EXP = mybir.ActivationFunctionType.Exp
    scale = 1.0 / np.sqrt(HD)

    with TileContext(nc) as tc:
        with (
            tc.tile_pool(name="const", bufs=1) as const,
            tc.tile_pool(name="big", bufs=1) as big,
            tc.tile_pool(name="xt", bufs=12) as xtp,
            tc.tile_pool(name="vsb", bufs=3) as vsbp,
            tc.tile_pool(name="ep", bufs=8) as ep,
            tc.tile_pool(name="e2p", bufs=6) as e2p,
            tc.tile_pool(name="work", bufs=4) as work,
            tc.tile_pool(name="obp", bufs=6) as obp,
            tc.tile_pool(name="accps", bufs=3, space="PSUM") as accps,
            tc.tile_pool(name="strps", bufs=5, space="PSUM") as strps,
            contextlib.ExitStack() as _loop_ctx,
        ):
            def phase_loop(p):
                if loop_n is not None and (loop_phase is None or loop_phase == p):
                    return tc.For_i(0, loop_n, 1)
                return contextlib.nullcontext()

            # ---- constants / weights ------------------------------------
            wq = []
            for d in range(ND):
                w = const.tile([128, 3 * DSL], F32R, name=f"wq{d}")
                nc.sync.dma_start(out=w, in_=WQKVT[d * 128:(d + 1) * 128, :])
                wq.append(w)
            woutt = const.tile([DSL, D], F32R, name="woutt")
            nc.sync.dma_start(out=woutt, in_=WOUTT[:, :])
            bias = []
            for f in range(3):
                bf = const.tile([DSL, 1], F32, name=f"bias{f}")
                nc.sync.dma_start(out=bf, in_=BQKV[f * DSL:(f + 1) * DSL, :])
                bias.append(bf)
            ident_f = const.tile([128, 128], F32, name="ident_f")
            make_identity(nc, ident_f)
            ident = const.tile([128, 128], F32R, name="ident")
            nc.vector.tensor_copy(ident, ident_f)
            ones_f = const.tile([128, 1], F32, name="ones_f")
            nc.vector.memset(ones_f, 1.0)
            ones2_f = const.tile([1, HD], F32, name="ones2_f")
            nc.vector.memset(ones2_f, 1.0)
            onesr = const.tile([1, HD], F32R, name="onesr")
            nc.vector.tensor_copy(onesr, ones2_f)
            # diagonal causal-mask tiles: mask[j][kk, qq] = (qq - kk - 128j >= 0)
            masks = []
            for j in range(4):
                mk = const.tile([128, 512], F32, name=f"mask{j}")
                nc.gpsimd.memset(mk, 1.0)
                nc.gpsimd.affine_select(
                    out=mk, in_=mk, compare_op=mybir.AluOpType.is_ge,
                    fill=0.0, base=-128 * j, channel_multiplier=-1,
                    pattern=[[1, 512]],
                )
                masks.append(mk)

            # ---- persistent activations ---------------------------------
            # Q^T, K^T feature-major (two heads stacked on partitions)
            qkvt = [big.tile([128, T], F32R, name=f"qkvt{f}") for f in range(2)]
            # V token-major + ones column, per (k-tile, head)
            vaug = big.tile([128, (T // 128) * 2 * (HD + 1)], F32R, name="vaug")
            attnt = big.tile([128, T], F32R, name="attnt")
            # all 64 ones-columns (offset HD, stride HD+1) in one strided copy
            nones = (T // 128) * 2
            ones64_f = const.tile([128, 64], F32, name="ones64_f")
            nc.vector.memset(ones64_f, 1.0)
            import concourse.ap as _ap
            vaug_ones_view = bass.AP(
                vaug.tensor, HD * 4, [[128, 128], [(HD + 1) * 4, nones]]
            ) if False else vaug[:, HD::HD + 1]
            nc.vector.tensor_copy(vaug_ones_view, ones64_f[:, 0:nones])

            # ---- work-item emitters -------------------------------------
            def emit_xt_dmas(sb2):
                xt = []
                for d in range(ND):
                    x = xtp.tile([128, 1024], F32R, name="xtc", tag="xt")
                    nc.sync.dma_start(
                        out=x,
                        in_=XT[d * 128:(d + 1) * 128,
                               sb2 * 1024:(sb2 + 1) * 1024],
                    )
                    xt.append(x)
                return xt

            def emit_qkv_group(xt, sb2, th, f):
                """One [128, 512] projection tile: 8 matmuls + bias add.
                For V (f == 2) also transpose to token-major into vaug."""
                t = sb2 * 2 + th
                ps = strps.tile([128, 512], F32, name="ps_qkv", tag="s")
                for d in range(ND):
                    nc.tensor.matmul(
                        ps,
                        wq[d][:, f * DSL:(f + 1) * DSL],
                        xt[d][:, th * 512:(th + 1) * 512],
                        start=(d == 0),
                        stop=(d == ND - 1),
                    )
                if f < 2:
                    nc.vector.tensor_scalar_add(
                        qkvt[f][:, t * 512:(t + 1) * 512], ps, bias[f]
                    )
                    return
                vsb = vsbp.tile([128, 512], F32R, name="vsb", tag="v")
                nc.vector.tensor_scalar_add(vsb, ps, bias[2])
                for i in range(4):
                    kt = t * 4 + i
                    tp = strps.tile([128, 128], F32R, name="ps_t", tag="s")
                    with nc.allow_low_precision(reason="transpose is a permutation"):
                        nc.tensor.transpose(
                            tp, vsb[:, i * 128:(i + 1) * 128], ident
                        )
                    for h in range(2):
                        base = (kt * 2 + h) * (HD + 1)
                        nc.vector.tensor_copy(
                            vaug[:, base:base + HD], tp[:, h * HD:(h + 1) * HD]
                        )

            def emit_scores(b, qb, kt, h):
                """scores^T + exp (+ causal mask on diagonal tiles).
                Returns (tile, column offset) for attn@V. For diagonal tile
                j = kt - 4*qb, query columns < 128*j are entirely masked, so
                everything runs on the [off:512] slice (off capped at 256:
                below 256 moving columns f32r matmuls drop to 1/4 rate, so
                narrower slices would not be faster)."""
                qc = b * S + qb * 512
                ktg = b * NKT + kt
                off = 0
                diag = kt >= 4 * qb
                if diag:
                    off = min(128 * (kt - 4 * qb), 256)
                ps_s = strps.tile([128, 512], F32, name="ps_s", tag="s")
                nc.tensor.matmul(
                    ps_s[:, off:512],
                    qkvt[1][h * HD:(h + 1) * HD, ktg * 128:(ktg + 1) * 128],
                    qkvt[0][h * HD:(h + 1) * HD, qc + off:qc + 512],
                    start=True, stop=True, tile_position=(h * HD, 0),
                )
                expt = ep.tile([128, 512], F32R, name="expt", tag="e")
                nc.scalar.activation(
                    expt[:, off:512], ps_s[:, off:512], EXP, scale=scale
                )
                if diag:
                    expt2 = e2p.tile([128, 512], F32R, name="expt2", tag="e2")
                    nc.vector.tensor_mul(
                        expt2[:, off:512], expt[:, off:512],
                        masks[kt - 4 * qb][:, off:512],
                    )
                    return expt2, off
                return expt, off

            def emit_attnv(ps_o, b, qb, kt, h, src_off, nkt):
                src_tile, off = src_off
                ktg = b * NKT + kt
                va = vaug[:, (ktg * 2 + h) * (HD + 1):
                          (ktg * 2 + h + 1) * (HD + 1)]
                nc.tensor.matmul(
                    ps_o[h][:, off:512], va, src_tile[:, off:512],
                    start=(kt == 0), stop=(kt == nkt - 1),
                )

            def make_epilogue(ps_o, b, qb):
                def epi():
                    qc = b * S + qb * 512
                    for h in range(2):
                        recip = work.tile([1, 512], F32R, name="recip", tag="r")
                        with nc.allow_low_precision(reason="softmax denom"):
                            nc.vector.reciprocal(recip, ps_o[h][HD:HD + 1, :])
                        ps_b = strps.tile([HD, 512], F32, name="ps_b", tag="s")
                        nc.tensor.matmul(ps_b, onesr, recip, start=True, stop=True)
                        bc = work.tile([HD, 512], F32, name="bc", tag="bc")
                        nc.vector.tensor_copy(bc, ps_b)
                        nc.vector.tensor_mul(
                            attnt[h * HD:(h + 1) * HD, qc:qc + 512],
                            ps_o[h][0:HD, :], bc,
                        )
                return epi

            in_tail = [False]  # True once all exps are emitted (flush)

            def make_outproj(b, qb):
                def opj():
                    tb = b * S + qb * 512
                    for e in range(ND):
                        ps = strps.tile([128, 512], F32, name="ps_out", tag="s")
                        nc.tensor.matmul(
                            ps,
                            woutt[:, e * 128:(e + 1) * 128],
                            attnt[:, tb:tb + 512],
                            start=True, stop=True,
                        )
                        ob = obp.tile([128, 512], F32, name="ob", tag="ob")
                        # In the flush tail ACT has no more exps to run, so
                        # split the PSUM evacuation between DVE and ACT
                        # (one activation-table switch total).
                        if in_tail[0] and e % 2 == 1:
                            nc.scalar.copy(ob, ps)
                        else:
                            nc.vector.tensor_copy(ob, ps)
                        alt = nc.gpsimd if loop_n is None else nc.scalar
                        eng = nc.sync if e % 2 == 0 else alt
                        eng.dma_start(
                            out=OUTT[e * 128:(e + 1) * 128, tb:tb + 512],
                            in_=ob,
                        )
                return opj

            # ---- interleaved emission -----------------------------------
            # QKV work for super-blocks 2..3 is threaded into batch 0's
            # attention so the PE fills exp-wait gaps with projection
            # matmuls and the X^T DMAs overlap attention compute.
            with phase_loop(0):
                qkv_items = []  # deferred (sb2 >= 2) qkv groups

                def run_qkv_sb(sb2, defer):
                    xt = emit_xt_dmas(sb2)
                    for th in range(2):
                        for f in range(3):
                            if defer:
                                qkv_items.append(
                                    (lambda xt=xt, sb2=sb2, th=th, f=f:
                                     emit_qkv_group(xt, sb2, th, f))
                                )
                            else:
                                emit_qkv_group(xt, sb2, th, f)

                run_qkv_sb(0, defer=False)
                run_qkv_sb(1, defer=False)
                run_qkv_sb(2, defer=True)
                run_qkv_sb(3, defer=True)

                pending = []
                toggle = [False]

                def pop_item():
                    # epilogues first: releases ps_o accumulator banks sooner
                    if pending:
                        pending.pop(0)()
                    elif qkv_items:
                        qkv_items.pop(0)()

                for b in range(B):
                    for qb in range(NQB):
                        nkt = 4 * qb + 4
                        ps_o = [
                            accps.tile([HD + 1, 512], F32,
                                       name=f"ps_o{h}", tag="o")
                            for h in range(2)
                        ]
                        # 2-deep lookahead: scores/exp run two k-tiles
                        # ahead of the attn@V consuming them, covering the
                        # exp+mask latency with two matmul groups.
                        DEPTH = 3
                        window = []
                        for kt0 in range(min(DEPTH, nkt)):
                            window.append(
                                [emit_scores(b, qb, kt0, h) for h in range(2)]
                            )
                            if kt0 == 0:
                                pop_item()
                        for kt in range(DEPTH, nkt):
                            cur = [emit_scores(b, qb, kt, h) for h in range(2)]
                            if kt % 2 == 1:
                                pop_item()
                            old_srcs = window.pop(0)
                            for h in range(2):
                                emit_attnv(ps_o, b, qb, kt - DEPTH, h, old_srcs[h], nkt)
                            window.append(cur)
                        base = max(0, nkt - DEPTH)
                        for j, srcs in enumerate(window):
                            for h in range(2):
                                emit_attnv(ps_o, b, qb, base + j, h, srcs[h], nkt)
                        pending.append(make_epilogue(ps_o, b, qb))
                        pending.append(make_outproj(b, qb))
                for fn in qkv_items:
                    fn()
                in_tail[0] = True
                for fn in pending:
                    fn()

    nc.finalize()
    _split_multi_waits(nc)
    return nc


_NC = None
LAST_EXEC_TIME_NS = None


def kernel(X, W_qkv, b_qkv, W_out, b_out):
    global _NC, LAST_EXEC_TIME_NS
    X = np.asarray(X, dtype=np.float32)
    W_qkv = np.asarray(W_qkv, dtype=np.float32)
    b_qkv = np.asarray(b_qkv, dtype=np.float32)
    W_out = np.asarray(W_out, dtype=np.float32)
    b_out = np.asarray(b_out, dtype=np.float32)

    XTv = np.ascontiguousarray(X.reshape(T, D).T)

    in_maps = []
    for c in range(NCORES):
        sl = slice(c * DSL, (c + 1) * DSL)
        wc = np.concatenate(
            [W_qkv[c * DSL:(c + 1) * DSL],
             W_qkv[D + c * DSL:D + (c + 1) * DSL],
             W_qkv[2 * D + c * DSL:2 * D + (c + 1) * DSL]],
            axis=0,
        )
        wqkvt = np.ascontiguousarray(wc.T)
        bq = np.concatenate(
            [b_qkv[sl], b_qkv[D + sl.start:D + sl.stop],
             b_qkv[2 * D + sl.start:2 * D + sl.stop]]
        ).reshape(3 * DSL, 1)
        woutt = np.ascontiguousarray(W_out[:, sl].T)
        in_maps.append(
            {
                "XT": XTv,
                "WQKVT": wqkvt,
                "BQKV": np.ascontiguousarray(bq),
                "WOUTT": woutt,
            }
        )

    if _NC is None:
        _NC = _build()
    res = run_bass_kernel_spmd(_NC, in_maps, core_ids=list(range(NCORES)))
    LAST_EXEC_TIME_NS = res.exec_time_ns

    total = res.results[0]["OUTT"].astype(np.float64)
    for r in res.results[1:]:
        total += r["OUTT"]
    out = total.T + b_out
    return np.ascontiguousarray(out.reshape(B, S, D).astype(np.float32))

